# revision 2
# baseline (speedup 1.0000x reference)
"""ColorHistogramLoss TRN2 kernel — x-space mixed-basis functional evaluation.

Reference math: hist[img,b] = sum_p exp(-(u_p-b)^2/C), u = 63*clip((x+1)/2,0,1),
loss = mean|normalize(hist_gen) - normalize(hist_tgt)|.

Strategy (v3): instead of evaluating all 64 (or 48) Gaussian centers per pixel
(the baseline: 48 ACT Exp passes + 48 DVE arg passes, ~100 us), evaluate K=43
basis FUNCTIONALS whose per-image sums are computed on device and from which
the 64 Gaussian histogram values are reconstructed host-side by a fixed linear
operator W (lstsq fit of the exact device functions to the Gaussian targets,
ridge-regularized so |W|~1 and fp32 accumulation noise is not amplified):

  ACT  (23 passes): erf(s_j*x + b_j) smooth steps evaluated on RAW x — the
       affine u=31.5x+31.5 folds into scale/bias, erf runs at Exp speed, and
       d/dc erf = gauss means M erf steps span ~ M Gaussians.  accum_out
       gives per-lane sums for free; no DVE involvement at all.
  DVE  (19 passes, fully parallel with ACT): one fused pass per functional
       with accum_out = sum:
         z = x*x (+sum z)        scalar_tensor_tensor(mult, mult)
         minq(s) = min(s*x, x^2) scalar_tensor_tensor(mult, min)
         ramp(c) = min(x, c)     tensor_scalar(min, add-reduce)
         step(c) = 1[x >= c]     tensor_scalar(is_ge, add-reduce)
       The clip kinks at x=+-1 are representable (knots there); no clip
       passes are needed anywhere.

Sharding: data-parallel over the 8 cores (H rows).  Per core the 12 images'
32768-pixel shards are laid out image-pure per lane:
  A-tile [128, 2048] = 8 images x 16 lanes, B-tile [128, 1024] = 4 x 32.
Host: sum lane groups -> S[12, K], append exact count column, hist = S @ W,
normalize + L1 (the reference formula verbatim).

Measured (axon TRN2): rel err vs reference 1.15e-4.  For_i-slope of the body:
94.5 us/iter vs the baseline body's 189.5 us/iter in the same harness (the
harness-reported baseline is 100110 ns) -> estimated ~50 us, ~2.0x speedup.
Engine balance: ACT ~53 us of erf passes, DVE ~55 us of fused passes, fully
overlapped; both engines ~100% busy, so further gains need fewer functionals
(error margin gets thin below K~40) or a third evaluation engine (PE/Pool
cannot do elementwise-nonlinearity+reduce on this HW).
"""

import math
import sys

for _p in ("/opt/trn_rl_repo",):
    if _p not in sys.path:
        sys.path.insert(0, _p)

from contextlib import ExitStack, nullcontext

import numpy as np

import concourse.bass as bass  # noqa: F401
import concourse.mybir as mybir
import concourse.tile as tile
from concourse import bacc
from concourse.bass_utils import run_bass_kernel_spmd

N_CORES = 8
B, C, H, W_IMG = 2, 3, 512, 512
N_IMG = B * C
NUM_BINS = 64
SIGMA_U = 63.0 * (1.5 / 64.0)
CDEN = 2.0 * SIGMA_U * SIGMA_U
ROWS_PER_CORE = H // N_CORES
PIX = ROWS_PER_CORE * W_IMG
F32 = mybir.dt.float32
BF16 = mybir.dt.bfloat16
ALU = mybir.AluOpType
AF = mybir.ActivationFunctionType

# ---------------------------------------------------------------------------
# Basis configuration (device-validated: rel err 1.15e-4 on the reference)
# ---------------------------------------------------------------------------
ERF_CENTERS_U = list(np.linspace(-1.0, 64.0, 23))
ERF_SIGMA_U = [1.35] * len(ERF_CENTERS_U)
MINQ_KNOTS_X = list(np.linspace(-0.9, 0.9, 8))
RAMP_KNOTS_X = list(np.linspace(-1.0, 1.0, 6))
STEP_KNOTS_X = [-1.0, -0.5, 0.0, 0.5]

K_ERF = len(ERF_CENTERS_U)
K_DVE = len(MINQ_KNOTS_X) + len(RAMP_KNOTS_X) + len(STEP_KNOTS_X) + 1  # + sz
K_DEV = K_ERF + K_DVE
K_TOT = K_DEV + 1  # + host count column

_erf_vec = np.vectorize(math.erf, otypes=[np.float64])


def _erf_coeffs():
    out = []
    for c, s in zip(ERF_CENTERS_U, ERF_SIGMA_U):
        sc = np.float32(31.5 / (np.sqrt(2.0) * s))
        bb = np.float32((31.5 - c) / (np.sqrt(2.0) * s))
        out.append((float(sc), float(bb)))
    return out


def basis_eval(x):
    """Exact model of the device functionals on fp32-exact raw x (float64)."""
    x = np.asarray(x, np.float64)
    z = (np.asarray(x, np.float32) * np.asarray(x, np.float32)).astype(np.float64)
    cols = []
    for sc, bb in _erf_coeffs():
        cols.append(_erf_vec(np.float64(sc) * x + np.float64(bb)))
    for s in MINQ_KNOTS_X:
        cols.append(np.minimum(np.float64(np.float32(s)) * x, z))
    for c in RAMP_KNOTS_X:
        cols.append(np.minimum(x, np.float64(np.float32(c))))
    for c in STEP_KNOTS_X:
        cols.append((x >= np.float64(np.float32(c))).astype(np.float64))
    cols.append(z)
    cols.append(np.ones_like(x))  # host count column
    return np.stack(cols, axis=-1)


def fit_W():
    grid = np.linspace(-5.4, 5.4, 8191)
    Bg = basis_eval(grid)
    u = np.clip(31.5 * grid + 31.5, 0.0, 63.0)
    bb = np.arange(64.0)
    G = np.exp(-((u[:, None] - bb[None, :]) ** 2) / CDEN)
    w = np.sqrt(np.exp(-0.5 * grid * grid) + 1e-4)
    A = w[:, None] * Bg
    Y = w[:, None] * G
    rows_a, rows_y = [], []
    for xx, wt in ((-1.5, 30.0), (1.5, 30.0), (-1.01, 20.0), (1.01, 20.0)):
        rows_a.append(wt * basis_eval(np.array([xx]))[0])
        uu = np.clip(31.5 * xx + 31.5, 0.0, 63.0)
        rows_y.append(wt * np.exp(-((uu - bb) ** 2) / CDEN))
    A = np.vstack([A] + rows_a)
    Y = np.vstack([Y] + rows_y)
    cs = np.linalg.norm(A, axis=0)
    cs[cs == 0] = 1.0
    An = A / cs
    K = An.shape[1]
    # ridge tuned on-device: large enough that |W|~1 (no fp32 accum-noise
    # amplification), small enough that the fit bias stays ~1e-4.
    Wn = np.linalg.solve(An.T @ An + 1e-4 * np.eye(K), An.T @ Y)
    return Wn / cs[:, None]


_CACHE: dict = {}


def _build_nc(reps: int = 0):
    """reps=0: the real kernel.  reps>0: wrap the functional block in a
    tc.For_i hardware loop for slope-timing (outputs are then garbage)."""
    nc = bacc.Bacc(
        "TRN2", target_bir_lowering=False, debug=False, enable_asserts=False
    )
    g6 = nc.dram_tensor("g6", [6, 16, 2048], F32, kind="ExternalInput")
    t01 = nc.dram_tensor("t01", [2, 16, 2048], F32, kind="ExternalInput")
    t25 = nc.dram_tensor("t25", [4, 32, 1024], F32, kind="ExternalInput")
    btab = nc.dram_tensor("btab", [128, max(K_ERF, 2)], F32, kind="ExternalInput")
    hist = nc.dram_tensor("hist", [256, K_DEV], F32, kind="ExternalOutput")

    coeffs = _erf_coeffs()

    with tile.TileContext(nc) as tc, ExitStack() as ctx:
        pool = ctx.enter_context(tc.tile_pool(name="main", bufs=1))
        wp = ctx.enter_context(tc.tile_pool(name="w", bufs=2))

        XA = pool.tile([128, 2048], F32, tag="xa")
        XB = pool.tile([128, 1024], F32, tag="xb")
        ZA = pool.tile([128, 2048], F32, tag="za")
        ZB = pool.tile([128, 1024], F32, tag="zb")
        HA = pool.tile([128, K_DEV], F32, tag="ha")
        HB = pool.tile([128, K_DEV], F32, tag="hb")
        BT = pool.tile([128, max(K_ERF, 2)], F32, tag="bt")

        # trigger the Erf table load ASAP so it overlaps the input DMAs
        dummy = pool.tile([128, 1], F32, tag="dummy")
        nc.scalar.activation(
            dummy[:], nc.const_aps.tensor(0.0, (128, 1)), AF.Erf,
            bias=0.0, scale=1.0,
        )

        nc.sync.dma_start(BT[:, :], btab.ap())
        nc.sync.dma_start(XB[:, :], t25.ap().rearrange("i s f -> (i s) f"))
        nc.sync.dma_start(XA[0:96, :], g6.ap().rearrange("i s f -> (i s) f"))
        nc.sync.dma_start(XA[96:128, :], t01.ap().rearrange("i s f -> (i s) f"))

        loop_cm = tc.For_i(0, reps) if reps > 0 else nullcontext()
        with loop_cm:
            _emit_body(nc, wp, coeffs, XA, XB, ZA, ZB, HA, HB, BT)

        nc.sync.dma_start(hist.ap()[128:256, :], HB[:])
        nc.sync.dma_start(hist.ap()[0:128, :], HA[:])
    nc.finalize()
    return nc


def _emit_body(nc, wp, coeffs, XA, XB, ZA, ZB, HA, HB, BT):
    iSZ = K_DEV - 1
    # z = x*x and sum(z) in one stt pass (the only DVE prep)
    for x_t, z_t, h_t in ((XB, ZB, HB), (XA, ZA, HA)):
        nc.vector.scalar_tensor_tensor(
            z_t[:], x_t[:], 1.0, x_t[:], ALU.mult, ALU.mult,
            accum_out=h_t[:, iSZ : iSZ + 1],
        )

    # ACT: erf steps on raw x (B tile first so ACT starts immediately)
    for j, (sc, bb) in enumerate(coeffs):
        for x_t, h_t, flen, tag in ((XB, HB, 1024, "b"), (XA, HA, 2048, "a")):
            w_t = wp.tile([128, flen], BF16, tag=f"we{tag}")
            nc.scalar.activation(
                w_t[:], x_t[:], AF.Erf,
                bias=BT[:, j : j + 1], scale=sc,
                accum_out=h_t[:, j : j + 1],
            )

    # DVE functionals (fp32, one fused pass each, parallel with ACT)
    col = K_ERF
    for s in MINQ_KNOTS_X:
        for x_t, z_t, h_t, flen, tag in (
            (XB, ZB, HB, 1024, "b"), (XA, ZA, HA, 2048, "a")
        ):
            w_t = wp.tile([128, flen], F32, tag=f"wv{tag}")
            nc.vector.scalar_tensor_tensor(
                w_t[:], x_t[:], float(np.float32(s)), z_t[:],
                ALU.mult, ALU.min, accum_out=h_t[:, col : col + 1],
            )
        col += 1
    for c in RAMP_KNOTS_X:
        for x_t, h_t, flen, tag in ((XB, HB, 1024, "b"), (XA, HA, 2048, "a")):
            w_t = wp.tile([128, flen], F32, tag=f"wv{tag}")
            nc.vector.tensor_scalar(
                w_t[:], x_t[:], float(np.float32(c)), 0.0,
                ALU.min, ALU.add, accum_out=h_t[:, col : col + 1],
            )
        col += 1
    for c in STEP_KNOTS_X:
        for x_t, h_t, flen, tag in ((XB, HB, 1024, "b"), (XA, HA, 2048, "a")):
            w_t = wp.tile([128, flen], F32, tag=f"wv{tag}")
            nc.vector.tensor_scalar(
                w_t[:], x_t[:], float(np.float32(c)), 0.0,
                ALU.is_ge, ALU.add, accum_out=h_t[:, col : col + 1],
            )
        col += 1
    assert col == K_DEV - 1


def _shard_inputs(generated: np.ndarray, target: np.ndarray):
    gen = np.ascontiguousarray(generated, dtype=np.float32).reshape(N_IMG, H, W_IMG)
    tgt = np.ascontiguousarray(target, dtype=np.float32).reshape(N_IMG, H, W_IMG)
    brow = np.zeros(max(K_ERF, 2), np.float32)
    for j, (sc, bb) in enumerate(_erf_coeffs()):
        brow[j] = np.float32(bb)
    btab = np.ascontiguousarray(np.broadcast_to(brow, (128, len(brow))))
    in_maps = []
    for cid in range(N_CORES):
        r0 = cid * ROWS_PER_CORE
        gs = gen[:, r0 : r0 + ROWS_PER_CORE, :].reshape(N_IMG, PIX)
        ts_ = tgt[:, r0 : r0 + ROWS_PER_CORE, :].reshape(N_IMG, PIX)
        in_maps.append(
            {
                "g6": np.ascontiguousarray(gs.reshape(6, 16, 2048)),
                "t01": np.ascontiguousarray(ts_[:2].reshape(2, 16, 2048)),
                "t25": np.ascontiguousarray(ts_[2:].reshape(4, 32, 1024)),
                "btab": btab,
            }
        )
    return in_maps


def _postprocess(per_core_hists) -> np.float32:
    S = np.zeros((12, K_TOT), np.float64)
    for h in per_core_hists:
        h = h.astype(np.float64)
        a = h[0:128].reshape(8, 16, K_DEV).sum(axis=1)    # gen0-5, tgt0-1
        bb = h[128:256].reshape(4, 32, K_DEV).sum(axis=1)  # tgt2-5
        S[0:8, :K_DEV] += a
        S[8:12, :K_DEV] += bb
    S[:, K_DEV] = PIX * N_CORES
    if "W" not in _CACHE:
        _CACHE["W"] = fit_W()
    Happ = S @ _CACHE["W"]
    hg = Happ[0:6]
    ht = Happ[6:12]
    hg = hg / (hg.sum(axis=-1, keepdims=True) + 1e-8)
    ht = ht / (ht.sum(axis=-1, keepdims=True) + 1e-8)
    return np.float32(np.mean(np.abs(hg - ht)))


def _run(in_maps, **kw):
    if "nc" not in _CACHE:
        _CACHE["nc"] = _build_nc()
    return run_bass_kernel_spmd(
        _CACHE["nc"], in_maps, core_ids=list(range(N_CORES)), **kw
    )


def kernel(generated: np.ndarray, target: np.ndarray) -> np.ndarray:
    generated = np.asarray(generated)
    target = np.asarray(target)
    assert generated.shape == (B, C, H, W_IMG) and target.shape == (B, C, H, W_IMG)
    in_maps = _shard_inputs(generated, target)
    res = _run(in_maps)
    return np.asarray(
        _postprocess([r["hist"] for r in res.results]), dtype=np.float32
    )
